# revision 1
# baseline (speedup 1.0000x reference)
"""Trainium2 Bass kernel for nn_AutoSparseLinear: out = sparse @ weight + b.

Shapes (hardcoded): sparse [4096, 4096] f32, weight [4096, 4096] f32,
b [4096] f32 -> out [4096, 4096] f32.

Strategy: data-parallel shard the batch dim across 8 cores (512 rows each).
Per core computes out_c^T = W^T @ x_c^T via PE matmuls with W tiles as the
stationary operand (so W streams from HBM exactly once) and x_c^T resident
in SBUF as the moving operand. Operands are cast to fp16 on the host:
fp16 runs the PE at 1 cycle/row and halves the weight DMA traffic vs fp32.
PSUM accumulation stays fp32. Bias is added during PSUM->SBUF eviction on
the vector engine ([128,1] per-partition scalar broadcast, since the
output-feature dim lands on partitions in the out^T layout); the eviction
writes bf16, which halves the output-store DMA traffic and leaves the
total absmax relative error at ~2.3e-3 (vs the 2e-2 budget).

DMA ring split: the 32 weight-tile DMAs (1 MiB each) go on the scalar
(ACT) HWDGE ring; x chunks, bias and output stores go on the sync (SP)
ring. Both rings are FIFO per issuing engine, so without the split the
first weight tile would queue behind the whole 4 MiB x load and the PE
would idle ~11 us at the start.

Host side only reshapes/transposes/casts for layout and concatenates
shards.
"""

import numpy as np

import concourse.bass as bass
import concourse.mybir as mybir
import concourse.tile as tile
from concourse import bacc
from concourse.bass_utils import run_bass_kernel_spmd

P = 128          # partitions
B = 4096         # full batch
NCORES = 8
M = B // NCORES  # batch rows per core = 512
K = 4096         # in_features (contract dim)
N = 4096         # out_features
KT = K // P      # 32 k-tiles
NT = N // P      # 32 n-tiles
XCH = 4          # x chunk DMAs
KPC = KT // XCH  # k-tiles per x chunk

MM_DT = mybir.dt.float16
NP_DT = np.float16
OUT_DT = mybir.dt.bfloat16

_CACHE = {}


def build_nc(repeat=1):
    nc = bacc.Bacc("TRN2", target_bir_lowering=False, debug=False)

    # xT[p, kt*M + m] = x_core[m, kt*P + p]   (moving operand)
    xT = nc.dram_tensor("xT", [P, KT * M], MM_DT,
                        kind="ExternalInput").ap()
    # w[nt, p, kt*P + j] = weight[kt*P + p, nt*P + j]  (stationary)
    w = nc.dram_tensor("w", [NT, P, KT * P], MM_DT,
                       kind="ExternalInput").ap()
    # bias[p, nt] = b[nt*P + p]
    bias = nc.dram_tensor("bias", [P, NT], mybir.dt.float32,
                          kind="ExternalInput").ap()
    # outT[nt, p, m] = out_core[m, nt*P + p]
    outT = nc.dram_tensor("outT", [NT, P, M], OUT_DT,
                          kind="ExternalOutput").ap()

    with tile.TileContext(nc) as tc:
        with (
            tc.tile_pool(name="xpool", bufs=1) as xpool,
            tc.tile_pool(name="wpool", bufs=4) as wpool,
            tc.tile_pool(name="opool", bufs=3) as opool,
            tc.tile_pool(name="bpool", bufs=1) as bpool,
            tc.tile_pool(name="pspool", bufs=4, space="PSUM") as pspool,
        ):
            bt = bpool.tile([P, NT], mybir.dt.float32)
            nc.sync.dma_start(bt[:], bias[:])

            # x^T resident in SBUF, loaded in 4 chunks so the first MM
            # groups can start while the tail of x is still in flight.
            xch = []
            for c in range(XCH):
                xc = xpool.tile([P, KPC * M], MM_DT, name=f"xc{c}",
                                tag=f"xc{c}")
                nc.sync.dma_start(xc[:],
                                  xT[:, c * KPC * M:(c + 1) * KPC * M])
                xch.append(xc)

            def xslice(kt):
                c, j = divmod(kt, KPC)
                return xch[c][:, j * M:(j + 1) * M]

            for r in range(repeat):
                for nt in range(NT):
                    wt = wpool.tile([P, KT * P], MM_DT, name=f"wt{r}_{nt}",
                                    tag="wt")
                    nc.scalar.dma_start(wt[:], w[nt])
                    ps = pspool.tile([P, M], mybir.dt.float32,
                                     name=f"ps{r}_{nt}", tag="ps")
                    for kt in range(KT):
                        nc.tensor.matmul(
                            ps[:],
                            wt[:, kt * P:(kt + 1) * P],
                            xslice(kt),
                            start=(kt == 0),
                            stop=(kt == KT - 1),
                        )
                    ot = opool.tile([P, M], OUT_DT, name=f"ot{r}_{nt}",
                                    tag="ot")
                    nc.vector.tensor_scalar_add(ot[:], ps[:],
                                                bt[:, nt:nt + 1])
                    nc.sync.dma_start(outT[nt], ot[:])

    nc.compile()
    return nc


def get_nc():
    if "nc" not in _CACHE:
        _CACHE["nc"] = build_nc()
    return _CACHE["nc"]


def shard_inputs(sparse, weight, b):
    sparse = np.asarray(sparse)
    weight = np.asarray(weight)
    b = np.ascontiguousarray(np.asarray(b), dtype=np.float32)

    # w[nt, p, kt*P + j] = weight[kt*P + p, nt*P + j]
    wb = np.ascontiguousarray(
        weight.astype(NP_DT).reshape(KT, P, NT, P).transpose(2, 1, 0, 3)
        .reshape(NT, P, KT * P)
    )
    bias_r = np.ascontiguousarray(b.reshape(NT, P).T)  # [P, NT]

    in_maps = []
    for c in range(NCORES):
        xs = sparse[c * M:(c + 1) * M, :].astype(NP_DT)  # [M, K]
        # xT[p, kt*M + m] = xs[m, kt*P + p]
        xb = np.ascontiguousarray(
            xs.reshape(M, KT, P).transpose(2, 1, 0).reshape(P, KT * M)
        )
        in_maps.append({"xT": xb, "w": wb, "bias": bias_r})
    return in_maps


def unshard_output(results):
    outs = []
    for c in range(NCORES):
        oT = results[c]["outT"].astype(np.float32)  # [NT, P, M]
        outs.append(oT.reshape(N, M).T)  # [M, N]
    return np.ascontiguousarray(np.concatenate(outs, axis=0))


def kernel(sparse, weight, b, **run_kwargs):
    nc = get_nc()
    in_maps = shard_inputs(sparse, weight, b)
    res = run_bass_kernel_spmd(nc, in_maps, core_ids=list(range(NCORES)),
                               **run_kwargs)
    out = unshard_output(res.results)
    if run_kwargs:
        _CACHE["last_result"] = res
    return out

